# revision 3
# baseline (speedup 1.0000x reference)
"""Trainium2 Bass kernel for CS-divergence loss (nn_CSDivergenceLoss).

Math: for diagonal 2-D Gaussians the pairwise overlap integral
  g_ij = (1/2pi) * exp(-0.5 * sum_d (m1-m2)^2/(v1+v2)) / sqrt(prod_d (v1+v2))
equals prod_d h_d(i,j) with h_d the 1-D Gaussian overlap integral
  h_d(i,j) = int N(x; m1_d, v1_d) N(x; m2_d, v2_d) dx.
Discretizing that integral with a trapezoid grid of Q=128 points makes h_d
SEPARABLE: h_d = sum_q phi_q(i) phi_q(j), phi_q(i) = sqrt(dx) N(x_q; m_i, v_i).
So each pair-sum  sum_ij w_ij g_ij  becomes elementwise products of three
PE matmuls:  W = A^T B (class weights), Hx = Phix^T Phix, Hy = Phiy^T Phiy,
and a weighted reduction. Rel. error of the quadrature is <= 2e-5 (validated
vs float64).

Sharding: data-parallel over batch; each of 8 cores handles 4 images and
emits its partial sum of (ln pp + ln qq - 2 ln pq); host adds 8 partials.

Feature matrices (input-sized, O(BS*K*Q)) are precomputed on host in numpy;
the O(K^2 * Q) work (matmuls + pairwise products + reductions) runs on
device.
"""

import math
from contextlib import ExitStack

import numpy as np

BS, KP, KG, NC = 32, 1000, 100, 80
Q = 128
GRID_LO, GRID_HI = -1.5, 2.5
N_CORES = 8
IMGS = BS // N_CORES  # images per core
PCH = 128             # partition chunk for the qq pair blocks
N_CHUNKS = (KP + PCH - 1) // PCH  # 8 (last chunk 104 rows)


# ----------------------------------------------------------------- host prep
def _log_sigmoid(x):
    # stable log(sigmoid(x)) = -log1p(exp(-x)) for x>0, x - log1p(exp(x)) else
    return np.where(x > 0, -np.log1p(np.exp(-x)), x - np.log1p(np.exp(x)))


def _features(m, v, lnscale=None):
    """phi[q, k] = exp(-(x_q-m_k)^2/(2 v_k) - 0.5*ln(2 pi v_k / dx) [+ lns_k])

    m, v: [..., K] float64. Returns [..., Q, K] float32.
    """
    grid = np.linspace(GRID_LO, GRID_HI, Q)
    dx = (GRID_HI - GRID_LO) / (Q - 1)
    d = grid[:, None] - m[..., None, :]                      # [..., Q, K]
    lognorm = -0.5 * np.log(2.0 * math.pi * v / dx)          # [..., K]
    arg = -0.5 * d * d / v[..., None, :] + lognorm[..., None, :]
    if lnscale is not None:
        arg = arg + lnscale[..., None, :]
    return np.exp(arg).astype(np.float32)


def _prep_host(pred_bboxes, pred_labels, gt_bboxes, gt_labels):
    pb = np.asarray(pred_bboxes, np.float64)
    pl = np.asarray(pred_labels, np.float64)
    gb = np.asarray(gt_bboxes, np.float64)
    gl = np.asarray(gt_labels)

    E = np.exp(pl[:, :, :NC])                                # [BS,KP,NC]
    lnscale = _log_sigmoid(pl[:, :, NC]) - np.log(E.sum(-1))  # [BS,KP]

    e_t = np.ascontiguousarray(E.transpose(0, 2, 1)).astype(np.float32)
    e2_t = (2.0 * e_t).astype(np.float32)                    # [BS,NC,KP]

    pm_x, pm_y = pb[:, :, 0], pb[:, :, 1]
    pv_x, pv_y = (pb[:, :, 2] / 2.0) ** 2, (pb[:, :, 3] / 2.0) ** 2
    gm_x, gm_y = gb[:, :, 0], gb[:, :, 1]
    gv_x, gv_y = (gb[:, :, 2] / 2.0) ** 2, (gb[:, :, 3] / 2.0) ** 2

    # softmax/sigmoid scale folded once into the pred x-dim features
    phix = _features(pm_x, pv_x, lnscale)                    # [BS,Q,KP]
    phiy = _features(pm_y, pv_y)
    gx = _features(gm_x, gv_x)                               # [BS,Q,KG]
    gy = _features(gm_y, gv_y)

    oht = np.zeros((BS, NC, KG), np.float32)                 # one-hot^T
    b_idx = np.repeat(np.arange(BS), KG)
    oht[b_idx, gl.reshape(-1).astype(np.int64), np.tile(np.arange(KG), BS)] = 1.0

    # per-image weight pattern for the device tail:
    # partial = sum_b (ln pp + ln qq - 2 ln pq);  stats cols = (pq, pp, qq) * 4
    wpat = np.tile(np.array([-2.0, 1.0, 1.0], np.float32), IMGS)[None, :]
    return dict(phix=phix, phiy=phiy, e=e_t, e2=e2_t, gx=gx, gy=gy, oht=oht,
                wpat=wpat)


# ------------------------------------------------------------- device program
_CACHE = {}


def _col_splits(lo, hi, bank=512):
    """Split [lo, hi) at multiples of `bank` (PSUM bank boundaries)."""
    out = []
    c = lo
    while c < hi:
        n = min(hi, (c // bank + 1) * bank) - c
        out.append((c, n))
        c += n
    return out


def build_program():
    if "nc" in _CACHE:
        return _CACHE["nc"]
    import concourse.bacc as bacc
    import concourse.tile as tile
    from concourse import mybir

    f32 = mybir.dt.float32
    MUL = mybir.AluOpType.mult

    nc = bacc.Bacc("TRN2", target_bir_lowering=False, debug=False,
                   num_devices=N_CORES)

    phix = nc.dram_tensor("phix", [IMGS, Q, KP], f32, kind="ExternalInput").ap()
    phiy = nc.dram_tensor("phiy", [IMGS, Q, KP], f32, kind="ExternalInput").ap()
    e1d = nc.dram_tensor("e", [IMGS, NC, KP], f32, kind="ExternalInput").ap()
    e2d = nc.dram_tensor("e2", [IMGS, NC, KP], f32, kind="ExternalInput").ap()
    gxd = nc.dram_tensor("gx", [IMGS, Q, KG], f32, kind="ExternalInput").ap()
    gyd = nc.dram_tensor("gy", [IMGS, Q, KG], f32, kind="ExternalInput").ap()
    ohtd = nc.dram_tensor("oht", [IMGS, NC, KG], f32, kind="ExternalInput").ap()
    wpatd = nc.dram_tensor("wpat", [1, 3 * IMGS], f32, kind="ExternalInput").ap()
    outp = nc.dram_tensor("partial", [1, 1], f32, kind="ExternalOutput").ap()
    outs = nc.dram_tensor("stats", [1, 3 * IMGS], f32, kind="ExternalOutput").ap()

    with tile.TileContext(nc) as tc, ExitStack() as ctx:
        const = ctx.enter_context(tc.tile_pool(name="const", bufs=1))
        feats = ctx.enter_context(tc.tile_pool(name="feats", bufs=2))
        work = ctx.enter_context(tc.tile_pool(name="work", bufs=3))
        hyb = ctx.enter_context(tc.tile_pool(name="hyb", bufs=2))
        ps_hx = ctx.enter_context(tc.tile_pool(name="ps_hx", bufs=1, space="PSUM"))
        ps_hy = ctx.enter_context(tc.tile_pool(name="ps_hy", bufs=1, space="PSUM"))
        ps_w = ctx.enter_context(tc.tile_pool(name="ps_w", bufs=1, space="PSUM"))
        ps_row = ctx.enter_context(tc.tile_pool(name="ps_row", bufs=1, space="PSUM"))

        ones = const.tile([PCH, 1], f32)
        nc.vector.memset(ones, 1.0)
        wpat_sb = const.tile([1, 3 * IMGS], f32)
        nc.sync.dma_start(wpat_sb, wpatd)
        stats = const.tile([1, 3 * IMGS], f32)
        scratch = const.tile([1, KP], f32)

        def pair_block(lhs_x, lhs_y, lhs_w, rows, rhs_x, rhs_y, w_segs,
                       row_ps, out_lo, start, stop):
            """One [rows, width] pair block.

            lhs_*: SBUF lhsT APs ([K, rows]); rhs_x/rhs_y: SBUF rhs APs
            ([K, width]); w_segs: list of (local_off, n, rhs_w_ap) for the
            class-weight matmul; row_ps: [1, KP] PSUM row accumulator;
            out_lo: absolute column of local col 0.
            """
            width = rhs_x.shape[-1]
            hx = ps_hx.tile([PCH, KP], f32, tag="hx")
            hy = ps_hy.tile([PCH, KP], f32, tag="hy")
            wt = ps_w.tile([PCH, KP], f32, tag="wt")
            for off, n in _col_splits(0, width):
                nc.tensor.matmul(hx[:rows, off:off + n], lhs_x,
                                 rhs_x[:, off:off + n], start=True, stop=True)
                nc.tensor.matmul(hy[:rows, off:off + n], lhs_y,
                                 rhs_y[:, off:off + n], start=True, stop=True)
            for off, n, rhs_w in w_segs:
                nc.tensor.matmul(wt[:rows, off:off + n], lhs_w, rhs_w,
                                 start=True, stop=True)
            hysb = hyb.tile([PCH, KP], f32, tag="hysb")
            nc.scalar.copy(hysb[:rows, :width], hy[:rows, :width])
            g = work.tile([PCH, KP], f32, tag="g")
            nc.vector.tensor_tensor(g[:rows, :width], hx[:rows, :width],
                                    hysb[:rows, :width], op=MUL)
            m = work.tile([PCH, KP], f32, tag="m")
            nc.vector.tensor_tensor(m[:rows, :width], g[:rows, :width],
                                    wt[:rows, :width], op=MUL)
            # ones-matvec: row_ps[0, out_lo + local] += sum_rows m
            for off, n in _col_splits(out_lo, out_lo + width):
                loff = off - out_lo
                nc.tensor.matmul(row_ps[0:1, off:off + n], ones[:rows, :],
                                 m[:rows, loff:loff + n], start=start,
                                 stop=stop, skip_group_check=True)

        for b in range(IMGS):
            px = feats.tile([Q, KP], f32, tag="px")
            nc.sync.dma_start(px, phix[b])
            py = feats.tile([Q, KP], f32, tag="py")
            nc.sync.dma_start(py, phiy[b])
            e1 = feats.tile([NC, KP], f32, tag="e1")
            nc.sync.dma_start(e1, e1d[b])
            e2 = feats.tile([NC, KP], f32, tag="e2")
            nc.sync.dma_start(e2, e2d[b])
            gxt = feats.tile([Q, KG], f32, tag="gx")
            nc.sync.dma_start(gxt, gxd[b])
            gyt = feats.tile([Q, KG], f32, tag="gy")
            nc.sync.dma_start(gyt, gyd[b])
            oht = feats.tile([NC, KG], f32, tag="oht")
            nc.sync.dma_start(oht, ohtd[b])

            # ---- qq: upper-triangular chunk blocks; off-diagonal doubled
            # via E2 so total = 2*sum_offdiag + sum_diag.
            qrow = ps_row.tile([1, KP], f32, tag="row")
            for c in range(N_CHUNKS):
                s = PCH * c
                rows = min(PCH, KP - s)
                width = KP - s
                # class-weight rhs: diag block from E, off-diag from 2E
                w_segs = [(0, rows, e1[:, s:s + rows])]
                for off, n in _col_splits(rows, width):
                    w_segs.append((off, n, e2[:, s + off:s + off + n]))
                pair_block(px[:, s:s + rows], py[:, s:s + rows],
                           e1[:, s:s + rows], rows,
                           px[:, s:], py[:, s:], w_segs, qrow, s,
                           start=(c == 0), stop=(c == N_CHUNKS - 1))
            nc.scalar.activation(scratch[:, :KP], qrow[0:1, :],
                                 func=_identity(),
                                 accum_out=stats[0:1, 3 * b + 2:3 * b + 3])

            # ---- pq: [KG, KP]
            prow = ps_row.tile([1, KP], f32, tag="row")
            w_segs = [(off, n, e1[:, off:off + n]) for off, n in _col_splits(0, KP)]
            pair_block(gxt[:, :], gyt[:, :], oht[:, :], KG,
                       px[:, :], py[:, :], w_segs, prow, 0,
                       start=True, stop=True)
            nc.scalar.activation(scratch[:, :KP], prow[0:1, :],
                                 func=_identity(),
                                 accum_out=stats[0:1, 3 * b:3 * b + 1])

            # ---- pp: [KG, KG]
            gwrow = ps_row.tile([1, KP], f32, tag="row")
            pair_block(gxt[:, :], gyt[:, :], oht[:, :], KG,
                       gxt[:, :], gyt[:, :], [(0, KG, oht[:, :])], gwrow, 0,
                       start=True, stop=True)
            nc.scalar.activation(scratch[:, :KG], gwrow[0:1, :KG],
                                 func=_identity(),
                                 accum_out=stats[0:1, 3 * b + 1:3 * b + 2])

        # ---- tail: partial = sum(wpat * ln(stats))
        lnrow = const.tile([1, 3 * IMGS], f32)
        nc.scalar.activation(lnrow, stats, func=_ln())
        wl = const.tile([1, 3 * IMGS], f32)
        nc.vector.tensor_tensor(wl, lnrow, wpat_sb, op=MUL)
        part = const.tile([1, 1], f32)
        nc.vector.reduce_sum(part, wl, axis=_axis_x())
        nc.sync.dma_start(outp, part)
        nc.sync.dma_start(outs, stats)

    nc.compile()
    _CACHE["nc"] = nc
    return nc


def _identity():
    from concourse import mybir
    return mybir.ActivationFunctionType.Identity


def _ln():
    from concourse import mybir
    return mybir.ActivationFunctionType.Ln


def _axis_x():
    from concourse import mybir
    return mybir.AxisListType.X


# ----------------------------------------------------------------- entrypoint
def kernel(pred_bboxes, pred_labels, gt_bboxes, gt_labels):
    from concourse.bass_utils import run_bass_kernel_spmd

    host = _prep_host(pred_bboxes, pred_labels, gt_bboxes, gt_labels)
    nc = build_program()

    in_maps = []
    for k in range(N_CORES):
        sl = slice(k * IMGS, (k + 1) * IMGS)
        in_maps.append({
            "phix": np.ascontiguousarray(host["phix"][sl]),
            "phiy": np.ascontiguousarray(host["phiy"][sl]),
            "e": np.ascontiguousarray(host["e"][sl]),
            "e2": np.ascontiguousarray(host["e2"][sl]),
            "gx": np.ascontiguousarray(host["gx"][sl]),
            "gy": np.ascontiguousarray(host["gy"][sl]),
            "oht": np.ascontiguousarray(host["oht"][sl]),
            "wpat": host["wpat"],
        })

    res = run_bass_kernel_spmd(nc, in_maps, list(range(N_CORES)))
    total = 0.0
    for r in res.results:
        total += float(r["partial"].reshape(-1)[0])
    return np.float32(total)


# revision 6
# speedup vs baseline: 1.9181x; 1.9181x over previous
"""Trainium2 Bass kernel for CS-divergence loss (nn_CSDivergenceLoss).

Math: for diagonal 2-D Gaussians the pairwise overlap integral
  g_ij = (1/2pi) * exp(-0.5 * sum_d (m1-m2)^2/(v1+v2)) / sqrt(prod_d (v1+v2))
equals prod_d h_d(i,j) with h_d the 1-D Gaussian overlap integral
  h_d(i,j) = int N(x; m1_d, v1_d) N(x; m2_d, v2_d) dx.
Discretizing that integral with a trapezoid grid of Q=128 points makes h_d
SEPARABLE: h_d = sum_q phi_q(i) phi_q(j), phi_q(i) = sqrt(dx) N(x_q; m_i, v_i).
So each pair-sum  sum_ij w_ij g_ij  becomes elementwise products of three
PE matmuls:  W = A^T B (class weights), Hx = Phix^T Phix, Hy = Phiy^T Phiy,
and a weighted reduction. Rel. error of the quadrature is <= 2e-5 (validated
vs float64).

Sharding: data-parallel over batch; each of 8 cores handles 4 images and
emits its partial sum of (ln pp + ln qq - 2 ln pq); host adds 8 partials.

Feature matrices (input-sized, O(BS*K*Q)) are precomputed on host in numpy;
the O(K^2 * Q) work (matmuls + pairwise products + reductions) runs on
device.
"""

import math
from contextlib import ExitStack

import numpy as np

BS, KP, KG, NC = 32, 1000, 100, 80
Q = 128
GRID_LO, GRID_HI = -1.5, 2.5
N_CORES = 8
IMGS = BS // N_CORES  # images per core
PCH = 128             # partition chunk for the qq pair blocks
N_CHUNKS = (KP + PCH - 1) // PCH  # 8 (last chunk 104 rows)


# ----------------------------------------------------------------- host prep
def _log_sigmoid(x):
    # stable log(sigmoid(x)) = -log1p(exp(-x)) for x>0, x - log1p(exp(x)) else
    return np.where(x > 0, -np.log1p(np.exp(-x)), x - np.log1p(np.exp(x)))


def _features(m, v, lnscale=None):
    """phi[q, k] = exp(-(x_q-m_k)^2/(2 v_k) - 0.5*ln(2 pi v_k / dx) [+ lns_k])

    m, v: [..., K] float64. Returns [..., Q, K] float32.
    """
    grid = np.linspace(GRID_LO, GRID_HI, Q)
    dx = (GRID_HI - GRID_LO) / (Q - 1)
    d = grid[:, None] - m[..., None, :]                      # [..., Q, K]
    lognorm = -0.5 * np.log(2.0 * math.pi * v / dx)          # [..., K]
    arg = -0.5 * d * d / v[..., None, :] + lognorm[..., None, :]
    if lnscale is not None:
        arg = arg + lnscale[..., None, :]
    return np.exp(arg).astype(np.float32)


def _prep_host(pred_bboxes, pred_labels, gt_bboxes, gt_labels):
    pb = np.asarray(pred_bboxes, np.float64)
    pl = np.asarray(pred_labels, np.float64)
    gb = np.asarray(gt_bboxes, np.float64)
    gl = np.asarray(gt_labels)

    E = np.exp(pl[:, :, :NC])                                # [BS,KP,NC]
    lnscale = _log_sigmoid(pl[:, :, NC]) - np.log(E.sum(-1))  # [BS,KP]

    e_t = np.ascontiguousarray(E.transpose(0, 2, 1)).astype(np.float32)
    e2_t = (2.0 * e_t).astype(np.float32)                    # [BS,NC,KP]

    pm_x, pm_y = pb[:, :, 0], pb[:, :, 1]
    pv_x, pv_y = (pb[:, :, 2] / 2.0) ** 2, (pb[:, :, 3] / 2.0) ** 2
    gm_x, gm_y = gb[:, :, 0], gb[:, :, 1]
    gv_x, gv_y = (gb[:, :, 2] / 2.0) ** 2, (gb[:, :, 3] / 2.0) ** 2

    # softmax/sigmoid scale folded once into the pred x-dim features
    phix = _features(pm_x, pv_x, lnscale)                    # [BS,Q,KP]
    phiy = _features(pm_y, pv_y)
    gx = _features(gm_x, gv_x)                               # [BS,Q,KG]
    gy = _features(gm_y, gv_y)

    oht = np.zeros((BS, NC, KG), np.float32)                 # one-hot^T
    b_idx = np.repeat(np.arange(BS), KG)
    oht[b_idx, gl.reshape(-1).astype(np.int64), np.tile(np.arange(KG), BS)] = 1.0

    # per-image weight pattern for the device tail:
    # partial = sum_b (ln pp + ln qq - 2 ln pq);  stats cols = (pq, pp, qq) * 4
    wpat = np.tile(np.array([-2.0, 1.0, 1.0], np.float32), IMGS)[None, :]
    return dict(phix=phix, phiy=phiy, e=e_t, e2=e2_t, gx=gx, gy=gy, oht=oht,
                wpat=wpat)


# ------------------------------------------------------------- device program
_CACHE = {}


def _col_splits(lo, hi, bank=512):
    """Split [lo, hi) at multiples of `bank` (PSUM bank boundaries)."""
    out = []
    c = lo
    while c < hi:
        n = min(hi, (c // bank + 1) * bank) - c
        out.append((c, n))
        c += n
    return out


def build_program():
    if "nc" in _CACHE:
        return _CACHE["nc"]
    import concourse.bacc as bacc
    import concourse.tile as tile
    from concourse import mybir

    f32 = mybir.dt.float32
    f32r = mybir.dt.float32r  # fp32 bits, full-rate PE feed (N>=256)
    MUL = mybir.AluOpType.mult

    def r(ap):
        return ap.bitcast(f32r)

    nc = bacc.Bacc("TRN2", target_bir_lowering=False, debug=False,
                   num_devices=N_CORES)

    phix = nc.dram_tensor("phix", [IMGS, Q, KP], f32, kind="ExternalInput").ap()
    phiy = nc.dram_tensor("phiy", [IMGS, Q, KP], f32, kind="ExternalInput").ap()
    e1d = nc.dram_tensor("e", [IMGS, NC, KP], f32, kind="ExternalInput").ap()
    e2d = nc.dram_tensor("e2", [IMGS, NC, KP], f32, kind="ExternalInput").ap()
    gxd = nc.dram_tensor("gx", [IMGS, Q, KG], f32, kind="ExternalInput").ap()
    gyd = nc.dram_tensor("gy", [IMGS, Q, KG], f32, kind="ExternalInput").ap()
    ohtd = nc.dram_tensor("oht", [IMGS, NC, KG], f32, kind="ExternalInput").ap()
    wpatd = nc.dram_tensor("wpat", [1, 3 * IMGS], f32, kind="ExternalInput").ap()
    outp = nc.dram_tensor("partial", [1, 1], f32, kind="ExternalOutput").ap()
    outs = nc.dram_tensor("stats", [1, 3 * IMGS], f32, kind="ExternalOutput").ap()

    with tile.TileContext(nc) as tc, ExitStack() as ctx:
        const = ctx.enter_context(tc.tile_pool(name="const", bufs=1))
        feats = ctx.enter_context(tc.tile_pool(name="feats", bufs=2))
        work = ctx.enter_context(tc.tile_pool(name="work", bufs=3))
        hyb = ctx.enter_context(tc.tile_pool(name="hyb", bufs=2))
        ps_hx = ctx.enter_context(tc.tile_pool(name="ps_hx", bufs=1, space="PSUM"))
        ps_hy = ctx.enter_context(tc.tile_pool(name="ps_hy", bufs=1, space="PSUM"))
        ps_w = ctx.enter_context(tc.tile_pool(name="ps_w", bufs=1, space="PSUM"))
        ps_row = ctx.enter_context(tc.tile_pool(name="ps_row", bufs=1, space="PSUM"))

        ones = const.tile([PCH, 1], f32)
        nc.vector.memset(ones, 1.0)
        wpat_sb = const.tile([1, 3 * IMGS], f32)
        nc.sync.dma_start(wpat_sb, wpatd)
        stats = const.tile([1, 3 * IMGS], f32)
        scratch = const.tile([1, KP], f32)

        def pair_block(lhs_x, lhs_y, lhs_w, rows, rhs_x, rhs_y, w_segs,
                       row_ps, out_lo, start, stop):
            """One [rows, width] pair block.

            lhs_*: SBUF lhsT APs ([K, rows]); rhs_x/rhs_y: SBUF rhs APs
            ([K, width]); w_segs: list of (local_off, n, rhs_w_ap) for the
            class-weight matmul; row_ps: [1, KP] PSUM row accumulator;
            out_lo: absolute column of local col 0.
            """
            width = rhs_x.shape[-1]
            hx = ps_hx.tile([PCH, KP], f32, tag="hx")
            hy = ps_hy.tile([PCH, KP], f32, tag="hy")
            wt = ps_w.tile([PCH, KP], f32, tag="wt")
            for off, n in _col_splits(0, width):
                nc.tensor.matmul(hx[:rows, off:off + n], r(lhs_x),
                                 r(rhs_x[:, off:off + n]), start=True, stop=True)
                nc.tensor.matmul(hy[:rows, off:off + n], r(lhs_y),
                                 r(rhs_y[:, off:off + n]), start=True, stop=True)
            for off, n, rhs_w in w_segs:
                nc.tensor.matmul(wt[:rows, off:off + n], r(lhs_w), r(rhs_w),
                                 start=True, stop=True)
            hysb = hyb.tile([PCH, KP], f32, tag="hysb")
            nc.scalar.copy(hysb[:rows, :width], hy[:rows, :width])
            g = work.tile([PCH, KP], f32, tag="g")
            nc.vector.tensor_tensor(g[:rows, :width], hx[:rows, :width],
                                    hysb[:rows, :width], op=MUL)
            m = work.tile([PCH, KP], f32, tag="m")
            nc.vector.tensor_tensor(m[:rows, :width], g[:rows, :width],
                                    wt[:rows, :width], op=MUL)
            # ones-matvec: row_ps[0, out_lo + local] += sum_rows m
            for off, n in _col_splits(out_lo, out_lo + width):
                loff = off - out_lo
                nc.tensor.matmul(row_ps[0:1, off:off + n], r(ones[:rows, :]),
                                 r(m[:rows, loff:loff + n]), start=start,
                                 stop=stop, skip_group_check=True)

        for b in range(IMGS):
            px = feats.tile([Q, KP], f32, tag="px")
            nc.sync.dma_start(px, phix[b])
            py = feats.tile([Q, KP], f32, tag="py")
            nc.sync.dma_start(py, phiy[b])
            e1 = feats.tile([NC, KP], f32, tag="e1")
            nc.sync.dma_start(e1, e1d[b])
            e2 = feats.tile([NC, KP], f32, tag="e2")
            nc.sync.dma_start(e2, e2d[b])
            gxt = feats.tile([Q, KG], f32, tag="gx")
            nc.sync.dma_start(gxt, gxd[b])
            gyt = feats.tile([Q, KG], f32, tag="gy")
            nc.sync.dma_start(gyt, gyd[b])
            oht = feats.tile([NC, KG], f32, tag="oht")
            nc.sync.dma_start(oht, ohtd[b])

            # ---- qq: upper-triangular chunk blocks; off-diagonal doubled
            # via E2 so total = 2*sum_offdiag + sum_diag.
            qrow = ps_row.tile([1, KP], f32, tag="row")
            for c in range(N_CHUNKS):
                s = PCH * c
                rows = min(PCH, KP - s)
                width = KP - s
                # class-weight rhs: diag block from E, off-diag from 2E
                w_segs = [(0, rows, e1[:, s:s + rows])]
                for off, n in _col_splits(rows, width):
                    w_segs.append((off, n, e2[:, s + off:s + off + n]))
                pair_block(px[:, s:s + rows], py[:, s:s + rows],
                           e1[:, s:s + rows], rows,
                           px[:, s:], py[:, s:], w_segs, qrow, s,
                           start=(c == 0), stop=(c == N_CHUNKS - 1))
            nc.scalar.activation(scratch[:, :KP], qrow[0:1, :],
                                 func=_identity(),
                                 accum_out=stats[0:1, 3 * b + 2:3 * b + 3])

            # ---- pq: [KG, KP]
            prow = ps_row.tile([1, KP], f32, tag="row")
            w_segs = [(off, n, e1[:, off:off + n]) for off, n in _col_splits(0, KP)]
            pair_block(gxt[:, :], gyt[:, :], oht[:, :], KG,
                       px[:, :], py[:, :], w_segs, prow, 0,
                       start=True, stop=True)
            nc.scalar.activation(scratch[:, :KP], prow[0:1, :],
                                 func=_identity(),
                                 accum_out=stats[0:1, 3 * b:3 * b + 1])

            # ---- pp: [KG, KG]
            gwrow = ps_row.tile([1, KP], f32, tag="row")
            pair_block(gxt[:, :], gyt[:, :], oht[:, :], KG,
                       gxt[:, :], gyt[:, :], [(0, KG, oht[:, :])], gwrow, 0,
                       start=True, stop=True)
            nc.scalar.activation(scratch[:, :KG], gwrow[0:1, :KG],
                                 func=_identity(),
                                 accum_out=stats[0:1, 3 * b + 1:3 * b + 2])

        # ---- tail: partial = sum(wpat * ln(stats))
        lnrow = const.tile([1, 3 * IMGS], f32)
        nc.scalar.activation(lnrow, stats, func=_ln())
        wl = const.tile([1, 3 * IMGS], f32)
        nc.vector.tensor_tensor(wl, lnrow, wpat_sb, op=MUL)
        part = const.tile([1, 1], f32)
        nc.vector.reduce_sum(part, wl, axis=_axis_x())
        nc.sync.dma_start(outp, part)
        nc.sync.dma_start(outs, stats)

    nc.compile()
    _CACHE["nc"] = nc
    return nc


def _identity():
    from concourse import mybir
    return mybir.ActivationFunctionType.Identity


def _ln():
    from concourse import mybir
    return mybir.ActivationFunctionType.Ln


def _axis_x():
    from concourse import mybir
    return mybir.AxisListType.X


# ----------------------------------------------------------------- entrypoint
def kernel(pred_bboxes, pred_labels, gt_bboxes, gt_labels):
    from concourse.bass_utils import run_bass_kernel_spmd

    host = _prep_host(pred_bboxes, pred_labels, gt_bboxes, gt_labels)
    nc = build_program()

    in_maps = []
    for k in range(N_CORES):
        sl = slice(k * IMGS, (k + 1) * IMGS)
        in_maps.append({
            "phix": np.ascontiguousarray(host["phix"][sl]),
            "phiy": np.ascontiguousarray(host["phiy"][sl]),
            "e": np.ascontiguousarray(host["e"][sl]),
            "e2": np.ascontiguousarray(host["e2"][sl]),
            "gx": np.ascontiguousarray(host["gx"][sl]),
            "gy": np.ascontiguousarray(host["gy"][sl]),
            "oht": np.ascontiguousarray(host["oht"][sl]),
            "wpat": host["wpat"],
        })

    res = run_bass_kernel_spmd(nc, in_maps, list(range(N_CORES)))
    total = 0.0
    for r in res.results:
        total += float(r["partial"].reshape(-1)[0])
    return np.float32(total)


# revision 22
# speedup vs baseline: 2.3334x; 1.2165x over previous
"""Trainium2 Bass kernel for CS-divergence loss (nn_CSDivergenceLoss).

Math: for diagonal 2-D Gaussians the pairwise overlap integral
  g_ij = (1/2pi) * exp(-0.5 * sum_d (m1-m2)^2/(v1+v2)) / sqrt(prod_d (v1+v2))
equals prod_d h_d(i,j) with h_d the 1-D Gaussian overlap integral
  h_d(i,j) = int N(x; m1_d, v1_d) N(x; m2_d, v2_d) dx.
Discretizing that integral with a trapezoid grid of Q=128 points makes h_d
SEPARABLE: h_d = sum_q phi_q(i) phi_q(j), phi_q(i) = sqrt(dx) N(x_q; m_i, v_i).
So each pair-sum  sum_ij w_ij g_ij  becomes elementwise products of three
PE matmuls:  W = A^T B (class weights), Hx = Phix^T Phix, Hy = Phiy^T Phiy,
and a weighted reduction. Rel. error of the quadrature is <= 2e-5 (validated
vs float64).

Sharding: data-parallel over batch; each of 8 cores handles 4 images and
emits its partial sum of (ln pp + ln qq - 2 ln pq); host adds 8 partials.

Feature matrices (input-sized, O(BS*K*Q)) are precomputed on host in numpy;
the O(K^2 * Q) work (matmuls + pairwise products + reductions) runs on
device.
"""

import math
from contextlib import ExitStack

import numpy as np

BS, KP, KG, NC = 32, 1000, 100, 80
Q = 128
GRID_LO, GRID_HI = -1.5, 2.5
N_CORES = 8
IMGS = BS // N_CORES  # images per core
PCH = 128             # partition chunk for the qq pair blocks
N_CHUNKS = (KP + PCH - 1) // PCH  # 8 (last chunk 104 rows)


# ----------------------------------------------------------------- host prep
def _log_sigmoid(x):
    # stable log(sigmoid(x)) = -log1p(exp(-x)) for x>0, x - log1p(exp(x)) else
    return np.where(x > 0, -np.log1p(np.exp(-x)), x - np.log1p(np.exp(x)))


def _features(m, v, lnscale=None):
    """phi[q, k] = exp(-(x_q-m_k)^2/(2 v_k) - 0.5*ln(2 pi v_k / dx) [+ lns_k])

    m, v: [..., K] float64. Returns [..., Q, K] float32.
    """
    grid = np.linspace(GRID_LO, GRID_HI, Q)
    dx = (GRID_HI - GRID_LO) / (Q - 1)
    d = grid[:, None] - m[..., None, :]                      # [..., Q, K]
    lognorm = -0.5 * np.log(2.0 * math.pi * v / dx)          # [..., K]
    arg = -0.5 * d * d / v[..., None, :] + lognorm[..., None, :]
    if lnscale is not None:
        arg = arg + lnscale[..., None, :]
    return np.exp(arg).astype(np.float32)


def _prep_host(pred_bboxes, pred_labels, gt_bboxes, gt_labels):
    pb = np.asarray(pred_bboxes, np.float64)
    pl = np.asarray(pred_labels, np.float64)
    gb = np.asarray(gt_bboxes, np.float64)
    gl = np.asarray(gt_labels)

    E = np.exp(pl[:, :, :NC])                                # [BS,KP,NC]
    lnscale = _log_sigmoid(pl[:, :, NC]) - np.log(E.sum(-1))  # [BS,KP]

    import ml_dtypes
    bf16 = ml_dtypes.bfloat16
    e_t = np.ascontiguousarray(E.transpose(0, 2, 1)).astype(bf16)
    e2_t = (2.0 * e_t.astype(np.float32)).astype(bf16)       # [BS,NC,KP]

    pm_x, pm_y = pb[:, :, 0], pb[:, :, 1]
    pv_x, pv_y = (pb[:, :, 2] / 2.0) ** 2, (pb[:, :, 3] / 2.0) ** 2
    gm_x, gm_y = gb[:, :, 0], gb[:, :, 1]
    gv_x, gv_y = (gb[:, :, 2] / 2.0) ** 2, (gb[:, :, 3] / 2.0) ** 2

    # softmax/sigmoid scale folded once into the pred x-dim features
    phix = _features(pm_x, pv_x, lnscale).astype(bf16)       # [BS,Q,KP]
    phiy = _features(pm_y, pv_y).astype(bf16)
    gx = _features(gm_x, gv_x).astype(bf16)                  # [BS,Q,KG]
    gy = _features(gm_y, gv_y).astype(bf16)

    oht = np.zeros((BS, NC, KG), bf16)                       # one-hot^T
    b_idx = np.repeat(np.arange(BS), KG)
    oht[b_idx, gl.reshape(-1).astype(np.int64), np.tile(np.arange(KG), BS)] = 1.0

    # per-image weight pattern for the device tail:
    # partial = sum_b (ln pp + ln qq - 2 ln pq);  stats cols = (pq, pp, qq) * 4
    wpat = np.tile(np.array([-2.0, 1.0, 1.0], np.float32), IMGS)[None, :]
    return dict(phix=phix, phiy=phiy, e=e_t, e2=e2_t, gx=gx, gy=gy, oht=oht,
                wpat=wpat)


# ------------------------------------------------------------- device program
_CACHE = {}


def _col_splits(lo, hi, bank=512):
    """Split [lo, hi) at multiples of `bank` (PSUM bank boundaries)."""
    out = []
    c = lo
    while c < hi:
        n = min(hi, (c // bank + 1) * bank) - c
        out.append((c, n))
        c += n
    return out


def build_program():
    if "nc" in _CACHE:
        return _CACHE["nc"]
    import concourse.bacc as bacc
    import concourse.tile as tile
    from concourse import mybir

    f32 = mybir.dt.float32
    bf16 = mybir.dt.bfloat16
    f32r = mybir.dt.float32r
    MUL = mybir.AluOpType.mult
    IDENT = mybir.ActivationFunctionType.Identity

    nc = bacc.Bacc("TRN2", target_bir_lowering=False, debug=False,
                   num_devices=N_CORES)

    phix = nc.dram_tensor("phix", [IMGS, Q, KP], bf16, kind="ExternalInput").ap()
    phiy = nc.dram_tensor("phiy", [IMGS, Q, KP], bf16, kind="ExternalInput").ap()
    e1d = nc.dram_tensor("e", [IMGS, NC, KP], bf16, kind="ExternalInput").ap()
    e2d = nc.dram_tensor("e2", [IMGS, NC, KP], bf16, kind="ExternalInput").ap()
    gxd = nc.dram_tensor("gx", [IMGS, Q, KG], bf16, kind="ExternalInput").ap()
    gyd = nc.dram_tensor("gy", [IMGS, Q, KG], bf16, kind="ExternalInput").ap()
    ohtd = nc.dram_tensor("oht", [IMGS, NC, KG], bf16, kind="ExternalInput").ap()
    wpatd = nc.dram_tensor("wpat", [1, 3 * IMGS], f32, kind="ExternalInput").ap()
    outp = nc.dram_tensor("partial", [1, 1], f32, kind="ExternalOutput").ap()
    outs = nc.dram_tensor("stats", [1, 3 * IMGS], f32, kind="ExternalOutput").ap()

    with tile.TileContext(nc) as tc, ExitStack() as ctx:
        const = ctx.enter_context(tc.tile_pool(name="const", bufs=1))
        feats = ctx.enter_context(tc.tile_pool(name="feats", bufs=2))
        work = ctx.enter_context(tc.tile_pool(name="work", bufs=3))
        stat_p = ctx.enter_context(tc.tile_pool(name="stat_p", bufs=2))
        ps_hx = ctx.enter_context(tc.tile_pool(name="ps_hx", bufs=2, space="PSUM"))
        ps_hy = ctx.enter_context(tc.tile_pool(name="ps_hy", bufs=2, space="PSUM"))
        ps_w = ctx.enter_context(tc.tile_pool(name="ps_w", bufs=2, space="PSUM"))
        ps_sm = ctx.enter_context(tc.tile_pool(name="ps_sm", bufs=2, space="PSUM"))

        wpat_sb = const.tile([1, 3 * IMGS], f32)
        nc.sync.dma_start(wpat_sb, wpatd)
        stats = const.tile([1, 3 * IMGS], f32)
        ones = const.tile([PCH, 1], f32)
        nc.vector.memset(ones, 1.0)

        seg_col = [0]  # running accumulator-column index (reset per image)

        def pair_block(lhs_x, lhs_y, rows, rhs_x, rhs_y, w_segs, st128):
            """One [rows, width] pair block, processed in 512-col segments
            so each PSUM tile is a single bank (enables double-buffering).

            w_segs: list of (local_off, n, lhsT_w, rhs_w) for the class
            weights.  Each segment's sum_cols(W*Hx*Hy) lands in its own
            column of st128 (index via seg_col).
            """
            width = rhs_x.shape[-1]
            for off, n in _col_splits(0, width):
                hx = ps_hx.tile([PCH, 512], f32, tag="hx")
                hy = ps_hy.tile([PCH, 512], f32, tag="hy")
                wt = ps_w.tile([PCH, 512], f32, tag="wt")
                nc.tensor.matmul(hx[:rows, :n], lhs_x, rhs_x[:, off:off + n],
                                 start=True, stop=True)
                nc.tensor.matmul(hy[:rows, :n], lhs_y, rhs_y[:, off:off + n],
                                 start=True, stop=True)
                for woff, wn, lhs_w, rhs_w in w_segs:
                    lo = max(woff, off)
                    hi = min(woff + wn, off + n)
                    if lo >= hi:
                        continue
                    nc.tensor.matmul(wt[:rows, lo - off:hi - off], lhs_w,
                                     rhs_w[:, lo - woff:hi - woff],
                                     start=True, stop=True)
                # HW: a DVE op may read at most ONE input from PSUM, so Hy
                # is staged to SBUF (bf16) by the otherwise-idle ACT engine.
                hysb = work.tile([PCH, 512], bf16, tag="hysb")
                nc.scalar.copy(hysb[:rows, :n], hy[:rows, :n])
                g = work.tile([PCH, 512], bf16, tag="g")
                nc.vector.tensor_tensor(g[:rows, :n], hx[:rows, :n],
                                        hysb[:rows, :n], op=MUL)
                m = work.tile([PCH, 512], bf16, tag="m")
                c = seg_col[0]
                seg_col[0] += 1
                nc.vector.scalar_tensor_tensor(m[:rows, :n], g[:rows, :n],
                                               1.0, wt[:rows, :n],
                                               op0=MUL, op1=MUL,
                                               accum_out=st128[:rows, c:c + 1])

        for b in range(IMGS):
            px = feats.tile([Q, KP], bf16, tag="px")
            nc.sync.dma_start(px, phix[b])
            py = feats.tile([Q, KP], bf16, tag="py")
            nc.sync.dma_start(py, phiy[b])
            e1 = feats.tile([NC, KP], bf16, tag="e1")
            nc.sync.dma_start(e1, e1d[b])
            e2 = feats.tile([NC, KP], bf16, tag="e2")
            nc.sync.dma_start(e2, e2d[b])
            gxt = feats.tile([Q, KG], bf16, tag="gx")
            nc.sync.dma_start(gxt, gxd[b])
            gyt = feats.tile([Q, KG], bf16, tag="gy")
            nc.sync.dma_start(gyt, gyd[b])
            oht = feats.tile([NC, KG], bf16, tag="oht")
            nc.sync.dma_start(oht, ohtd[b])

            # per-image per-partition accumulators, one column per segment:
            # qq segs -> cols 0..11, pq -> 12..13, pp -> 14
            st128 = stat_p.tile([PCH, 16], f32, tag="st128")
            nc.gpsimd.memset(st128, 0.0)
            seg_col[0] = 0

            # ---- qq: upper-triangular chunk blocks; off-diagonal doubled
            # via E2 so total = 2*sum_offdiag + sum_diag.
            for c in range(N_CHUNKS):
                s = PCH * c
                rows = min(PCH, KP - s)
                width = KP - s
                w_segs = [(0, rows, e1[:, s:s + rows], e1[:, s:s + rows])]
                if width > rows:
                    w_segs.append((rows, width - rows, e1[:, s:s + rows],
                                   e2[:, s + rows:]))
                pair_block(px[:, s:s + rows], py[:, s:s + rows], rows,
                           px[:, s:], py[:, s:], w_segs, st128)
            n_qq = seg_col[0]

            # ---- pq: [KG, KP]
            pair_block(gxt[:, :], gyt[:, :], KG, px[:, :], py[:, :],
                       [(0, KP, oht[:, :], e1[:, :])], st128)
            n_pq = seg_col[0]

            # ---- pp: [KG, KG]
            pair_block(gxt[:, :], gyt[:, :], KG, gxt[:, :], gyt[:, :],
                       [(0, KG, oht[:, :], oht[:, :])], st128)
            n_all = seg_col[0]

            # partition-reduce the per-image stats via a tiny ones-matvec
            srow = ps_sm.tile([1, 16], f32, tag="srow")
            nc.tensor.matmul(srow[0:1, 0:n_all], ones,
                             st128[:, 0:n_all], start=True, stop=True)
            scr2 = stat_p.tile([1, 16], f32, tag="scr2")
            nc.scalar.activation(scr2[0:1, 0:n_qq], srow[0:1, 0:n_qq],
                                 func=IDENT,
                                 accum_out=stats[0:1, 3 * b + 2:3 * b + 3])
            nc.scalar.activation(scr2[0:1, n_qq:n_pq], srow[0:1, n_qq:n_pq],
                                 func=IDENT,
                                 accum_out=stats[0:1, 3 * b:3 * b + 1])
            nc.scalar.activation(scr2[0:1, n_pq:n_all], srow[0:1, n_pq:n_all],
                                 func=IDENT,
                                 accum_out=stats[0:1, 3 * b + 1:3 * b + 2])

        # ---- tail: partial = sum(wpat * ln(stats))
        lnrow = const.tile([1, 3 * IMGS], f32)
        nc.scalar.activation(lnrow, stats, func=_ln())
        wl = const.tile([1, 3 * IMGS], f32)
        nc.vector.tensor_tensor(wl, lnrow, wpat_sb, op=MUL)
        part = const.tile([1, 1], f32)
        nc.vector.reduce_sum(part, wl, axis=_axis_x())
        nc.sync.dma_start(outp, part)
        nc.sync.dma_start(outs, stats)

    nc.compile()
    _CACHE["nc"] = nc
    return nc


def _identity():
    from concourse import mybir
    return mybir.ActivationFunctionType.Identity


def _ln():
    from concourse import mybir
    return mybir.ActivationFunctionType.Ln


def _axis_x():
    from concourse import mybir
    return mybir.AxisListType.X


# ----------------------------------------------------------------- entrypoint
def kernel(pred_bboxes, pred_labels, gt_bboxes, gt_labels):
    from concourse.bass_utils import run_bass_kernel_spmd

    host = _prep_host(pred_bboxes, pred_labels, gt_bboxes, gt_labels)
    nc = build_program()

    in_maps = []
    for k in range(N_CORES):
        sl = slice(k * IMGS, (k + 1) * IMGS)
        in_maps.append({
            "phix": np.ascontiguousarray(host["phix"][sl]),
            "phiy": np.ascontiguousarray(host["phiy"][sl]),
            "e": np.ascontiguousarray(host["e"][sl]),
            "e2": np.ascontiguousarray(host["e2"][sl]),
            "gx": np.ascontiguousarray(host["gx"][sl]),
            "gy": np.ascontiguousarray(host["gy"][sl]),
            "oht": np.ascontiguousarray(host["oht"][sl]),
            "wpat": host["wpat"],
        })

    res = run_bass_kernel_spmd(nc, in_maps, list(range(N_CORES)))
    total = 0.0
    for r in res.results:
        total += float(r["partial"].reshape(-1)[0])
    return np.float32(total)
